# revision 1
# baseline (speedup 1.0000x reference)
"""MaxSim ranker kernel for 8 Trainium2 NeuronCores.

Strategy (matches the sharding hint): shard the `vectors` table by doc range
across the 8 cores (625 docs each), replicate `q_vectors`/`token_ids`, give
each core a boundary-shifted `emb2pid` so all per-core programs are identical
(SPMD). Each core:
  1. gathers pids = emb2pid_shifted[token_ids] via indirect DMA,
  2. scatters an additive mask (0 for present local docs, -1e30 otherwise),
  3. streams its V^T shard (pre-split into bf16 hi/lo on host) through the
     PE as a 3-pass bf16x2 matmul (qh*vh + qh*vl + ql*vh ~ fp32 precision),
  4. max-reduces doc-token columns on DVE, sums over queries with a
     block-ones matmul, adds the mask,
  5. extracts local top-104 (13 rounds of max8/max_index/match_replace).
Host merges the 8 local top-k lists into the global top-k.
"""

import sys

for _p in ("/opt/trn_rl_repo", "/root/.axon_site/_ro/trn_rl_repo"):
    if _p not in sys.path:
        sys.path.append(_p)

import numpy as np
import ml_dtypes

# ---- problem constants (hardcoded per contract) ----
N_DOCS = 5000
DOC_LEN = 128
DIM = 128
B = 8
NQ = 32
NTOK = 1024
N_EMB = N_DOCS * DOC_LEN
NCORES = 8
SHARD = N_DOCS // NCORES          # 625 docs per core
SH_DOCS = 640                      # padded shard docs (80 groups x 8 docs)
COLS = SH_DOCS * DOC_LEN           # 80896 V^T columns per core
GROUP = 1024                       # matmul/reduce group: 8 docs
NGRP = COLS // GROUP               # 80
ROUNDS = 13                        # (legacy, unused)
CHUNK = 40                         # per-(row,chunk) doc count; top-CHUNK extracted fully
NEG = -1.0e30

_PROGRAM = None


def _build_program(repeat=1, passes=3, ablate_reduce=False, ablate_dma=False, ablate_mask=False, topk_rounds=ROUNDS, dma_groups=2, group=1024, psum_bufs=2, v_bufs=6):
    import concourse.bass as bass
    import concourse.mybir as mybir
    import concourse.tile as tile
    from concourse import bacc

    bf16 = mybir.dt.bfloat16
    f32 = mybir.dt.float32

    nc = bacc.Bacc("TRN2", target_bir_lowering=False, debug=False)

    f32r = mybir.dt.float32r
    if passes == 0:  # float32r single-pass experiment
        qw = nc.dram_tensor("qw", [DIM, 256], f32r, kind="ExternalInput")
        vh = nc.dram_tensor("vh", [DIM, COLS], f32r, kind="ExternalInput")
    else:
        qw = nc.dram_tensor("qw", [DIM, 512], bf16, kind="ExternalInput")
        vh = nc.dram_tensor("vh", [DIM, COLS], bf16, kind="ExternalInput")
        vl = nc.dram_tensor("vl", [DIM, COLS], bf16, kind="ExternalInput")
    # tokT[p, j] = token_ids.flat[j*128 + p] — row b occupies columns 8b..8b+8
    tokT = nc.dram_tensor("tokT", [128, NTOK * B // 128], mybir.dt.int32, kind="ExternalInput")
    e2p = nc.dram_tensor("e2p", [1, N_EMB], mybir.dt.int32, kind="ExternalInput")
    topv = nc.dram_tensor("topv", [128, CHUNK], f32, kind="ExternalOutput")
    topi = nc.dram_tensor("topi", [128, CHUNK], mybir.dt.uint32, kind="ExternalOutput")

    mask_dram = [nc.dram_tensor(f"maskd{b}", [NTOK, 1], f32) for b in range(B)]

    import contextlib

    with tile.TileContext(nc) as tc:
        rep_ctx = tc.For_i(0, repeat, 1) if repeat > 1 else contextlib.nullcontext()
        with (
            rep_ctx,
            tc.tile_pool(name="const", bufs=1) as cpool,
            tc.tile_pool(name="v", bufs=v_bufs) as vpool,
            tc.tile_pool(name="ps", bufs=psum_bufs, space="PSUM") as pspool,
            tc.tile_pool(name="res", bufs=1) as rpool,
        ):
            # ---- small constants / index plumbing ----
            qw_sb = cpool.tile([DIM, 512 if passes else 256], bf16 if passes else f32r)
            nc.sync.dma_start(qw_sb[:], qw[:])

            JCOLS = NTOK * B // 128  # 64
            if ablate_mask:
                mask_sb = cpool.tile([B, SH_DOCS], f32)
                nc.vector.memset(mask_sb[:], 0.0)
            tok_sb = cpool.tile([128, JCOLS], mybir.dt.int32)
            if not ablate_mask:
                nc.sync.dma_start(tok_sb[:], tokT[:])

            # HW indirect DMA consumes ONE offset per dest partition-row, so
            # gather/scatter go column-by-column: 128 scalars per instruction.
            pids_sb = cpool.tile([128, JCOLS], mybir.dt.int32)
            for j in range(JCOLS if not ablate_mask else 0):
                nc.gpsimd.indirect_dma_start(
                    out=pids_sb[:, j : j + 1],
                    out_offset=None,
                    in_=e2p[:],
                    in_offset=bass.IndirectOffsetOnAxis(
                        ap=tok_sb[:, j : j + 1], axis=1
                    ),
                )

            # Tile does not track dependencies through internal DRAM tensors,
            # so the init -> scatter -> reload chain needs explicit dep edges.
            from concourse.bass import _add_dep_helper

            negs = cpool.tile([1, NTOK], f32)
            nc.vector.memset(negs[:], NEG)
            init_insts = []
            for b in range(B if not ablate_mask else 0):
                init_insts.append(
                    nc.sync.dma_start(
                        mask_dram[b].ap().rearrange("n o -> o n"), negs[:]
                    )
                )

            zeros = cpool.tile([128, 1], f32)
            nc.vector.memset(zeros[:], 0.0)
            pids_u32 = pids_sb[:].bitcast(mybir.dt.uint32)
            scat_insts = [[] for _ in range(B)]
            for b in range(B if not ablate_mask else 0):
                for i in range(JCOLS // B):
                    j = (JCOLS // B) * b + i
                    si = nc.gpsimd.indirect_dma_start(
                        out=mask_dram[b][:],
                        out_offset=bass.IndirectOffsetOnAxis(
                            ap=pids_u32[:, j : j + 1], axis=0
                        ),
                        in_=zeros[:],
                        in_offset=None,
                        bounds_check=SHARD - 1,
                        oob_is_err=False,
                    )
                    _add_dep_helper(
                        si.ins,
                        init_insts[b].ins,
                        sync=True,
                        reason="scatter after mask init",
                    )
                    scat_insts[b].append(si)

            if not ablate_mask:
                mask_sb = cpool.tile([B, SH_DOCS], f32)
            nc.vector.memset(mask_sb[:], NEG if not ablate_mask else 0.0)
            for b in range(B if not ablate_mask else 0):
                ri = nc.sync.dma_start(
                    mask_sb[b : b + 1, :SHARD],
                    mask_dram[b].ap().rearrange("n o -> o n")[:, :SHARD],
                )
                for si in scat_insts[b]:
                    _add_dep_helper(
                        ri.ins, si.ins, sync=True, reason="reload after scatter"
                    )

            # ones weights for the q-sum matmul: col j sums batch j's 32 queries
            ones0 = cpool.tile([DIM, 8], f32)
            ones1 = cpool.tile([DIM, 8], f32)
            nc.vector.memset(ones0[:], 0.0)
            nc.vector.memset(ones1[:], 0.0)
            for j in range(4):
                nc.vector.memset(ones0[32 * j : 32 * j + 32, j : j + 1], 1.0)
                nc.vector.memset(ones1[32 * j : 32 * j + 32, 4 + j : 5 + j], 1.0)

            maxres0 = rpool.tile([128, SH_DOCS], f32)
            maxres1 = rpool.tile([128, SH_DOCS], f32)
            maxres = (maxres0, maxres1)

            # ---- main stream: 3-pass bf16x2 matmul + grouped max-reduce ----
            gdocs = group // DOC_LEN
            ngrp = COLS // group
            assert ngrp % dma_groups == 0
            for gc in range(ngrp // dma_groups):
                CH = group * dma_groups
                vh_t = vpool.tile([DIM, CH], bf16 if passes else f32r, tag="vh")
                if passes:
                    vl_t = vpool.tile([DIM, CH], bf16, tag="vl")
                if not ablate_dma:
                    nc.sync.dma_start(vh_t[:], vh[:, gc * CH : (gc + 1) * CH])
                    if passes:
                        nc.scalar.dma_start(vl_t[:], vl[:, gc * CH : (gc + 1) * CH])
                for gi in range(dma_groups):
                    g = gc * dma_groups + gi
                    for qc in range(2):
                        ps = pspool.tile([128, group], f32, tag=f"ps{qc}")
                        qh = qw_sb[:, 128 * qc : 128 * qc + 128]
                        if passes:
                            ql = qw_sb[:, 256 + 128 * qc : 256 + 128 * qc + 128]
                        for s in range(group // 512):
                            sl = slice(512 * s, 512 * (s + 1))
                            gsl = slice(gi * group + 512 * s, gi * group + 512 * (s + 1))
                            nc.tensor.matmul(ps[:, sl], qh, vh_t[:, gsl], start=True, stop=(passes != 3))
                            if passes == 3:
                                nc.tensor.matmul(ps[:, sl], qh, vl_t[:, gsl], start=False, stop=False)
                                nc.tensor.matmul(ps[:, sl], ql, vh_t[:, gsl], start=False, stop=True)
                        if not ablate_reduce:
                            nc.vector.tensor_reduce(
                                out=maxres[qc][:, g * gdocs : (g + 1) * gdocs],
                                in_=ps[:].rearrange("p (d t) -> p d t", t=DOC_LEN),
                                axis=mybir.AxisListType.X,
                                op=mybir.AluOpType.max,
                            )

            # ---- q-sum (block-ones matmul), mask, top-k ----
            ps_s = pspool.tile([B, SH_DOCS], f32, tag="ps0")
            for lo in range(0, SH_DOCS, 512):
                hi = min(lo + 512, SH_DOCS)
                nc.tensor.matmul(
                    ps_s[:, lo:hi], ones0[:, :B], maxres0[:, lo:hi], start=True, stop=False
                )
                nc.tensor.matmul(
                    ps_s[:, lo:hi], ones1[:, :B], maxres1[:, lo:hi], start=False, stop=True
                )

            work8 = rpool.tile([B, SH_DOCS], f32)
            nc.vector.tensor_add(out=work8[:], in0=ps_s[:], in1=mask_sb[:])

            # reshape [8, 640] -> [128, 40]: row b chunk c (40 docs) at partition 16b+c.
            # Per-chunk top-40 extraction (5 rounds of max8) is then complete and
            # data-independent: every unmasked doc appears in the output.
            work = rpool.tile([128, CHUNK], f32)
            nc.sync.dma_start(
                work[:],
                work8[:].rearrange("b (c d) -> b c d", d=CHUNK),
            )

            tv = rpool.tile([128, CHUNK], f32)
            ti = rpool.tile([128, CHUNK], mybir.dt.uint32)
            for r in range(CHUNK // 8):
                sl = slice(8 * r, 8 * r + 8)
                nc.vector.max(out=tv[:, sl], in_=work[:])
                nc.vector.max_index(out=ti[:, sl], in_max=tv[:, sl], in_values=work[:])
                nc.vector.match_replace(
                    out=work[:], in_to_replace=tv[:, sl], in_values=work[:], imm_value=NEG
                )
            nc.sync.dma_start(topv[:], tv[:])
            nc.sync.dma_start(topi[:], ti[:])

    nc.compile()
    return nc


def _get_program(repeat=1):
    global _PROGRAM
    if repeat != 1:
        return _build_program(repeat)
    if _PROGRAM is None:
        _PROGRAM = _build_program()
    return _PROGRAM


def _bf16_split(x):
    hi = x.astype(ml_dtypes.bfloat16)
    lo = (x - hi.astype(np.float32)).astype(ml_dtypes.bfloat16)
    return hi, lo


def _prepare_in_maps(q_vectors, token_ids, vectors, emb2pid):
    q = np.ascontiguousarray(np.asarray(q_vectors, dtype=np.float32))
    V = np.asarray(vectors, dtype=np.float32)
    tok = np.ascontiguousarray(np.asarray(token_ids).astype(np.int32))
    e2p = np.asarray(emb2pid).astype(np.int32)

    qt = np.ascontiguousarray(q.reshape(B * NQ, DIM).T)      # [128, 256]
    qh, ql = _bf16_split(qt)
    qw_np = np.concatenate([qh, ql], axis=1)                  # [128, 512]

    tokT = np.ascontiguousarray(tok.reshape(-1).reshape(B * NTOK // 128, 128).T)

    in_maps = []
    for c in range(NCORES):
        vs = V[c * SHARD : (c + 1) * SHARD]                   # [625, 128, 128]
        vt = vs.transpose(2, 0, 1).reshape(DIM, SHARD * DOC_LEN)
        vt_p = np.zeros((DIM, COLS), np.float32)
        vt_p[:, : SHARD * DOC_LEN] = vt
        vh_np, vl_np = _bf16_split(vt_p)
        in_maps.append(
            {
                "qw": qw_np,
                "vh": vh_np,
                "vl": vl_np,
                "tokT": tokT,
                "e2p": (e2p - np.int32(c * SHARD)).reshape(1, N_EMB),
            }
        )
    return in_maps


def _merge(results, k_val):
    top_scores = np.empty((B, k_val), np.float32)
    top_pids = np.empty((B, k_val), np.int32)
    # per core: topv/topi [128, CHUNK]; partition 16*b + cc -> row b, chunk cc
    nchunk = SH_DOCS // CHUNK  # 16
    vals = []   # per row: list of arrays
    pids = []
    all_v = [[] for _ in range(B)]
    all_i = [[] for _ in range(B)]
    for c in range(NCORES):
        tv = np.asarray(results[c]["topv"], np.float32).reshape(B, nchunk, CHUNK)
        ti = np.asarray(results[c]["topi"]).astype(np.int64).reshape(B, nchunk, CHUNK)
        base = c * SHARD + np.arange(nchunk)[:, None] * CHUNK  # [nchunk, 1]
        for b in range(B):
            all_v[b].append(tv[b].reshape(-1))
            all_i[b].append((ti[b] + base).reshape(-1))
    for b in range(B):
        v = np.concatenate(all_v[b])
        i = np.concatenate(all_i[b])
        valid = v > -1.0e29
        v = v[valid]
        i = i[valid]
        order = np.argsort(-v, kind="stable")[:k_val]
        top_scores[b] = v[order]
        top_pids[b] = i[order].astype(np.int32)
    return top_scores, top_pids


def _run(inputs, trace=False, trace_kwargs=None):
    from concourse.bass_utils import run_bass_kernel_spmd

    nc = _get_program()
    in_maps = _prepare_in_maps(
        inputs["q_vectors"], inputs["token_ids"], inputs["vectors"], inputs["emb2pid"]
    )
    br = run_bass_kernel_spmd(
        nc, in_maps, list(range(NCORES)), trace=trace, **(trace_kwargs or {})
    )
    k_val = int(np.asarray(inputs.get("k", 100)))
    outs = _merge(br.results, k_val)
    return outs, br


def kernel(q_vectors, token_ids, vectors, emb2pid, k=100):
    outs, _ = _run(
        {
            "q_vectors": q_vectors,
            "token_ids": token_ids,
            "vectors": vectors,
            "emb2pid": emb2pid,
            "k": k,
        }
    )
    return outs



# revision 5
# speedup vs baseline: 2.0900x; 2.0900x over previous
"""MaxSim ranker kernel for 8 Trainium2 NeuronCores.

Strategy (matches the sharding hint): shard the `vectors` table by doc range
across the 8 cores (625 docs each), replicate `q_vectors`. Host precomputes a
per-(core,batch) additive candidate mask (0 for docs hit by the batch's
token_ids, -1e30 otherwise) so the device skips the emb2pid gather/scatter.
Each core:
  1. streams its V^T shard (fp16 on host) through the PE: scores[q, col] for
     all 256 queries x 80896 doc-token columns (PASSES=1: single fp16 matmul;
     PASSES=3: fp16 hi/lo x2 for ~fp32-exact scores),
  2. max-reduces each doc's 128 token columns, splitting the work across
     three pipelines: DVE tensor_reduce straight from PSUM, ACT-copy->SBUF
     fp16 + DVE tensor_tensor max tree (2x mode), ACT-copy + GPSIMD max tree,
  3. sums over each batch's 32 queries with a block-ones matmul, adds the
     host mask,
  4. extracts per-40-doc-chunk full sort (5 rounds of max8/max_index/
     match_replace) -> topv/topi [128, 40].
Host merges the 8 local lists into the global top-k.
"""

import sys

for _p in ("/opt/trn_rl_repo", "/root/.axon_site/_ro/trn_rl_repo"):
    if _p not in sys.path:
        sys.path.append(_p)

import numpy as np

# ---- problem constants (hardcoded per contract) ----
N_DOCS = 5000
DOC_LEN = 128
DIM = 128
B = 8
NQ = 32
NTOK = 1024
N_EMB = N_DOCS * DOC_LEN
NCORES = 8
SHARD = N_DOCS // NCORES          # 625 docs per core
SH_DOCS = 640                      # padded shard docs
COLS = SH_DOCS * DOC_LEN           # 81920 V^T columns per core
GROUP = 1024                       # one PSUM tile: 8 docs x 128 tokens
SUPER = 4                          # groups per reduce unit (32 docs)
NCHUNK = COLS // (GROUP * SUPER)   # 20 V chunks of 4096 cols
CHUNK = 40                         # per-(row,chunk) doc count for topk export
NEG = -1.0e30

PASSES = 1                         # 1 = fp16 single pass; 3 = fp16 hi/lo exact

_PROGRAMS = {}


def _routing(n_units, n_a, n_b, n_c):
    """Interleave A/B/C unit assignments evenly across the stream."""
    assert n_a + n_b + n_c == n_units
    counts = {"A": n_a, "B": n_b, "C": n_c}
    acc = {"A": 0.0, "B": 0.0, "C": 0.0}
    out = []
    for _ in range(n_units):
        for k in acc:
            acc[k] += counts[k] / n_units
        pick = max(acc, key=lambda k: acc[k])
        acc[pick] -= 1.0
        out.append(pick)
    return out


def _build_program(passes=PASSES, n_a=None, n_b=None, n_c=None):
    import concourse.bass as bass
    import concourse.mybir as mybir
    import concourse.tile as tile
    from concourse import bacc

    f16 = mybir.dt.float16
    f32 = mybir.dt.float32

    nc = bacc.Bacc("TRN2", target_bir_lowering=False, debug=False)

    N_UNITS = NCHUNK * 2  # (chunk, qc) pairs of SUPER groups each
    if n_a is None:
        # defaults tuned from the engine cost model (C = gpsimd tree is
        # disabled: walrus rejects TensorTensor on Pool in this toolchain)
        if passes == 1:
            n_a, n_b, n_c = 8, 32, 0
        else:
            n_a, n_b, n_c = 6, 34, 0
    routes = _routing(N_UNITS, n_a, n_b, n_c)

    qw = nc.dram_tensor("qw", [DIM, 256 * (2 if passes == 3 else 1)], f16,
                        kind="ExternalInput")
    vh = nc.dram_tensor("vh", [DIM, COLS], f16, kind="ExternalInput")
    if passes == 3:
        vl = nc.dram_tensor("vl", [DIM, COLS], f16, kind="ExternalInput")
    maskd = nc.dram_tensor("maskd", [B, SH_DOCS], f32, kind="ExternalInput")
    topv = nc.dram_tensor("topv", [128, CHUNK], f32, kind="ExternalOutput")
    topi = nc.dram_tensor("topi", [128, CHUNK], mybir.dt.uint32, kind="ExternalOutput")

    # reduce result dtype: fp16 in fast mode (feeds fp16 ones-matmul), fp32 in
    # exact mode.
    rdt = f16 if passes == 1 else f32

    with tile.TileContext(nc) as tc:
        with (
            tc.tile_pool(name="const", bufs=1) as cpool,
            tc.tile_pool(name="v", bufs=3) as vpool,
            tc.tile_pool(name="ps", bufs=2, space="PSUM") as pspool,
            tc.tile_pool(name="stg", bufs=2) as spool,
            tc.tile_pool(name="tree", bufs=2) as tpool,
            tc.tile_pool(name="res", bufs=1) as rpool,
        ):
            qw_sb = cpool.tile([DIM, 256 * (2 if passes == 3 else 1)], f16)
            nc.sync.dma_start(qw_sb[:], qw[:])
            mask_sb = cpool.tile([B, SH_DOCS], f32)
            nc.sync.dma_start(mask_sb[:], maskd[:])

            # ones weights for the q-sum matmul: col j sums batch j's 32 queries
            ones0 = cpool.tile([DIM, 8], rdt)
            ones1 = cpool.tile([DIM, 8], rdt)
            nc.vector.memset(ones0[:], 0.0)
            nc.vector.memset(ones1[:], 0.0)
            for j in range(4):
                nc.vector.memset(ones0[32 * j : 32 * j + 32, j : j + 1], 1.0)
                nc.vector.memset(ones1[32 * j : 32 * j + 32, 4 + j : 5 + j], 1.0)

            maxres0 = rpool.tile([128, SH_DOCS], rdt)
            maxres1 = rpool.tile([128, SH_DOCS], rdt)
            maxres = (maxres0, maxres1)

            CH = GROUP * SUPER  # 4096 cols per chunk
            unit = 0
            for gc in range(NCHUNK):
                vh_t = vpool.tile([DIM, CH], f16, tag="vh")
                nc.sync.dma_start(vh_t[:], vh[:, gc * CH : (gc + 1) * CH])
                if passes == 3:
                    vl_t = vpool.tile([DIM, CH], f16, tag="vl")
                    nc.sync.dma_start(vl_t[:], vl[:, gc * CH : (gc + 1) * CH])
                for qc in range(2):
                    route = routes[unit]
                    unit += 1
                    qh = qw_sb[:, 128 * qc : 128 * qc + 128]
                    if passes == 3:
                        ql = qw_sb[:, 256 + 128 * qc : 256 + 128 * qc + 128]
                    doff = (gc * SUPER) * 8  # first doc of this chunk
                    if route != "A":
                        stg = spool.tile([128, CH], rdt, tag=f"stg{qc}")
                    ps_list = []
                    for gi in range(SUPER):
                        ps = pspool.tile([128, GROUP], f32, tag=f"ps{qc}")
                        ps_list.append(ps)
                        for s in range(GROUP // 512):
                            sl = slice(512 * s, 512 * (s + 1))
                            gsl = slice(gi * GROUP + 512 * s,
                                        gi * GROUP + 512 * (s + 1))
                            nc.tensor.matmul(ps[:, sl], qh, vh_t[:, gsl],
                                             start=True, stop=(passes == 1))
                            if passes == 3:
                                nc.tensor.matmul(ps[:, sl], ql, vh_t[:, gsl],
                                                 start=False, stop=False)
                                nc.tensor.matmul(ps[:, sl], qh, vl_t[:, gsl],
                                                 start=False, stop=True)
                        if route == "A":
                            nc.vector.tensor_reduce(
                                out=maxres[qc][:, doff + gi * 8 : doff + gi * 8 + 8],
                                in_=ps[:].rearrange("p (d t) -> p d t", t=DOC_LEN),
                                axis=mybir.AxisListType.X,
                                op=mybir.AluOpType.max,
                            )
                        else:
                            # ACT converts PSUM fp32 -> SBUF fp16
                            nc.scalar.copy(
                                out=stg[:, gi * GROUP : (gi + 1) * GROUP], in_=ps[:]
                            )
                    if route != "A":
                        # 7-level pairwise max tree over tokens: [128, 32, 128]
                        # -> [128, 32]; DVE gets the 2x 16-bit mode, GPSIMD is
                        # the overflow lane.
                        eng = nc.vector if route == "B" else nc.gpsimd
                        src = stg[:].rearrange("p (d t) -> p d t", t=DOC_LEN)
                        width = DOC_LEN
                        lvl = 0
                        while width > 1:
                            half = width // 2
                            if half == 1:
                                dst = maxres[qc][:, doff : doff + 32]
                                dst_v = dst.rearrange("p (d t) -> p d t", t=1)
                            else:
                                t_t = tpool.tile([128, 32 * half], rdt,
                                                 tag=f"t{route}{qc}l{lvl}")
                                dst_v = t_t[:].rearrange("p (d t) -> p d t", t=half)
                            eng.tensor_tensor(
                                out=dst_v,
                                in0=src[:, :, 0:half],
                                in1=src[:, :, half:width],
                                op=mybir.AluOpType.max,
                            )
                            src = dst_v
                            width = half
                            lvl += 1

            # ---- q-sum (block-ones matmul), mask, topk ----
            ps_s = pspool.tile([B, SH_DOCS], f32, tag="ps0")
            for lo in range(0, SH_DOCS, 512):
                hi = min(lo + 512, SH_DOCS)
                nc.tensor.matmul(ps_s[:, lo:hi], ones0[:, :B], maxres0[:, lo:hi],
                                 start=True, stop=False)
                nc.tensor.matmul(ps_s[:, lo:hi], ones1[:, :B], maxres1[:, lo:hi],
                                 start=False, stop=True)

            work8 = rpool.tile([B, SH_DOCS], f32)
            nc.vector.tensor_add(out=work8[:], in0=ps_s[:], in1=mask_sb[:])

            # reshape [8, 640] -> [128, 40]: row b chunk c (40 docs) at
            # partition 16b+c; per-chunk top-40 extraction is then complete.
            work = rpool.tile([128, CHUNK], f32)
            nc.sync.dma_start(
                work[:],
                work8[:].rearrange("b (c d) -> b c d", d=CHUNK),
            )

            tv = rpool.tile([128, CHUNK], f32)
            ti = rpool.tile([128, CHUNK], mybir.dt.uint32)
            for r in range(CHUNK // 8):
                sl = slice(8 * r, 8 * r + 8)
                nc.vector.max(out=tv[:, sl], in_=work[:])
                nc.vector.max_index(out=ti[:, sl], in_max=tv[:, sl], in_values=work[:])
                nc.vector.match_replace(
                    out=work[:], in_to_replace=tv[:, sl], in_values=work[:],
                    imm_value=NEG
                )
            nc.sync.dma_start(topv[:], tv[:])
            nc.sync.dma_start(topi[:], ti[:])

    nc.compile()
    return nc


def _get_program(**kw):
    key = tuple(sorted(kw.items()))
    if key not in _PROGRAMS:
        _PROGRAMS[key] = _build_program(**kw)
    return _PROGRAMS[key]


def _fp16_split(x):
    hi = x.astype(np.float16)
    lo = (x - hi.astype(np.float32)).astype(np.float16)
    return hi, lo


def _prepare_in_maps(q_vectors, token_ids, vectors, emb2pid, passes=PASSES):
    q = np.ascontiguousarray(np.asarray(q_vectors, dtype=np.float32))
    V = np.asarray(vectors, dtype=np.float32)
    tok = np.asarray(token_ids).astype(np.int64)
    e2p = np.asarray(emb2pid).astype(np.int64)

    qt = np.ascontiguousarray(q.reshape(B * NQ, DIM).T)      # [128, 256]
    if passes == 3:
        qh, ql = _fp16_split(qt)
        qw_np = np.concatenate([qh, ql], axis=1)             # [128, 512]
    else:
        qw_np = qt.astype(np.float16)

    # host-side candidate masks: pids hit per batch, mapped to local doc ids
    pids = e2p[np.clip(tok, 0, N_EMB - 1)]                   # [B, NTOK]
    pids = np.where((tok < 0) | (tok >= N_EMB), -1, pids)
    pids = np.where((pids < 0) | (pids >= N_DOCS), -1, pids)

    in_maps = []
    for c in range(NCORES):
        vs = V[c * SHARD : (c + 1) * SHARD]                  # [625, 128, 128]
        vt = vs.transpose(2, 0, 1).reshape(DIM, SHARD * DOC_LEN)
        vt_p = np.zeros((DIM, COLS), np.float32)
        vt_p[:, : SHARD * DOC_LEN] = vt
        mask = np.full((B, SH_DOCS), NEG, np.float32)
        lo, hi = c * SHARD, (c + 1) * SHARD
        for b in range(B):
            local = pids[b][(pids[b] >= lo) & (pids[b] < hi)] - lo
            mask[b, local] = 0.0
        m = {"qw": qw_np, "maskd": mask}
        if passes == 3:
            m["vh"], m["vl"] = _fp16_split(vt_p)
        else:
            m["vh"] = vt_p.astype(np.float16)
        in_maps.append(m)
    return in_maps


def _merge(results, k_val):
    top_scores = np.empty((B, k_val), np.float32)
    top_pids = np.empty((B, k_val), np.int32)
    nchunk = SH_DOCS // CHUNK  # 16
    all_v = [[] for _ in range(B)]
    all_i = [[] for _ in range(B)]
    for c in range(NCORES):
        tv = np.asarray(results[c]["topv"], np.float32).reshape(B, nchunk, CHUNK)
        ti = np.asarray(results[c]["topi"]).astype(np.int64).reshape(B, nchunk, CHUNK)
        base = c * SHARD + np.arange(nchunk)[:, None] * CHUNK  # [nchunk, 1]
        for b in range(B):
            all_v[b].append(tv[b].reshape(-1))
            all_i[b].append((ti[b] + base).reshape(-1))
    for b in range(B):
        v = np.concatenate(all_v[b])
        i = np.concatenate(all_i[b])
        valid = v > -1.0e29
        v = v[valid]
        i = i[valid]
        order = np.argsort(-v, kind="stable")[:k_val]
        top_scores[b] = v[order]
        top_pids[b] = i[order].astype(np.int32)
    return top_scores, top_pids


def _run(inputs, trace=False, trace_kwargs=None, program_kwargs=None):
    from concourse.bass_utils import run_bass_kernel_spmd

    pk = dict(program_kwargs or {})
    nc = _get_program(**pk)
    in_maps = _prepare_in_maps(
        inputs["q_vectors"], inputs["token_ids"], inputs["vectors"],
        inputs["emb2pid"], passes=pk.get("passes", PASSES)
    )
    br = run_bass_kernel_spmd(
        nc, in_maps, list(range(NCORES)), trace=trace, **(trace_kwargs or {})
    )
    k_val = int(np.asarray(inputs.get("k", 100)))
    outs = _merge(br.results, k_val)
    return outs, br


def kernel(q_vectors, token_ids, vectors, emb2pid, k=100):
    outs, _ = _run(
        {
            "q_vectors": q_vectors,
            "token_ids": token_ids,
            "vectors": vectors,
            "emb2pid": emb2pid,
            "k": k,
        }
    )
    return outs
